# revision 1
# baseline (speedup 1.0000x reference)
"""BarrierNet Trainium2 kernel: MLP (6->128->128x2 branches->heads) + closed-form QP.

Data-parallel over 8 cores (16384 samples each). Host pre-shards and
pre-transposes: xt [6,NS] bf16, weights pre-transposed bf16 (heads padded to
M=32 col-groups), and broadcast-constant pattern tiles. Per core:
  - MLP in transposed layout (hidden on partitions, batch free), 8 chunks of
    2048; each layer = 4 bf16 N=512 matmuls into a 4-bank PSUM group drained
    by ONE fused bias+tanh ACT op (fp32 PSUM -> bf16 SBUF).
  - Heads: 4 chunks' [3,512]/[2,512] outputs packed into one PSUM bank via
    tile_position col-groups -> cheap FD=512 DVE copies.
  - Head outputs round-trip through DRAM scratch into sample-grid layout
    [128, j] where the QP math runs batched full-width on DVE in fp32.

All DMAs issue from the (otherwise idle) Pool/GPSIMD engine via SWDGE.
"""
import sys

sys.path.insert(0, "/opt/trn_rl_repo")

import numpy as np
import ml_dtypes

import concourse.bacc as bacc
import concourse.bass as bass
import concourse.mybir as mybir
import concourse.tile as tile
from concourse import bass_utils

FP = mybir.dt.float32
BF = mybir.dt.bfloat16
AF = mybir.ActivationFunctionType
OP = mybir.AluOpType
BF_NP = ml_dtypes.bfloat16

N_CORES = 8
B = 131072
NS = B // N_CORES          # samples per core
SC = 2048                  # super-chunk (one PSUM group span)
H = 128
NF = 6

_cache = {}


def build(ns=NS):
    nit = ns // SC
    NQ = 2
    jh = ns // NQ // 128   # samples per partition in one QP grid
    J3, J2, J6 = 3 * jh, 2 * jh, 6 * jh

    nc = bacc.Bacc("TRN2", target_bir_lowering=False, debug=False)

    x_d = nc.dram_tensor("x", [ns, NF], FP, kind="ExternalInput")
    xt_d = nc.dram_tensor("xt", [NF, ns], BF, kind="ExternalInput")
    w1T_d = nc.dram_tensor("w1T", [NF, H], BF, kind="ExternalInput")
    w21T_d = nc.dram_tensor("w21T", [H, H], BF, kind="ExternalInput")
    w22T_d = nc.dram_tensor("w22T", [H, H], BF, kind="ExternalInput")
    wm1T_d = nc.dram_tensor("wm1T", [H, H], BF, kind="ExternalInput")
    wm2T_d = nc.dram_tensor("wm2T", [H, H], BF, kind="ExternalInput")
    wh31_d = nc.dram_tensor("wh31", [H, 32], BF, kind="ExternalInput")
    wh32_d = nc.dram_tensor("wh32", [H, 32], BF, kind="ExternalInput")
    b1_d = nc.dram_tensor("b1", [H], FP, kind="ExternalInput")
    b21_d = nc.dram_tensor("b21", [H], FP, kind="ExternalInput")
    b22_d = nc.dram_tensor("b22", [H], FP, kind="ExternalInput")
    bm1_d = nc.dram_tensor("bm1", [H], FP, kind="ExternalInput")
    bm2_d = nc.dram_tensor("bm2", [H], FP, kind="ExternalInput")
    # qc columns: 0:J6 stdg | J6:2*J6 moffg | 2*J6:2*J6+J3 b31g | +J2 b32g
    QW = 2 * J6 + J3 + J2
    qc_d = nc.dram_tensor("qc", [H, QW], FP, kind="ExternalInput")
    u_d = nc.dram_tensor("u", [ns, 3], FP, kind="ExternalOutput")

    dma = nc.gpsimd.dma_start

    with tile.TileContext(nc) as tc:
        with (
            tc.tile_pool(name="const", bufs=1) as cpool,
            tc.tile_pool(name="act", bufs=2) as apool,
            tc.tile_pool(name="hd", bufs=2) as hpool,
            tc.tile_pool(name="psum", bufs=2, space="PSUM") as ppool,
            tc.tile_pool(name="qp", bufs=2) as qpool,
            tc.tile_pool(name="qtmp", bufs=2) as tpool,
            tc.tile_pool(name="dram", bufs=1, space="DRAM") as dpool,
        ):
            # ---------------- load weights / constants (all contiguous) -----
            w1T = cpool.tile([NF, H], BF, tag="w1T", name="w1T")
            nc.sync.dma_start(w1T[:], w1T_d[:])
            wh31 = cpool.tile([H, 32], BF, tag="wh31", name="wh31")
            dma(wh31[:], wh31_d[:])
            wh32 = cpool.tile([H, 32], BF, tag="wh32", name="wh32")
            dma(wh32[:], wh32_d[:])
            wts = {}
            for nm, wd in (("w21", w21T_d), ("w22", w22T_d),
                           ("wm1", wm1T_d), ("wm2", wm2T_d)):
                wt = cpool.tile([H, H], BF, tag=nm + "T")
                dma(wt[:], wd[:])
                wts[nm] = wt
            bias = {}
            for nm, bd in (("b1", b1_d), ("b21", b21_d), ("b22", b22_d),
                           ("bm1", bm1_d), ("bm2", bm2_d)):
                bt = cpool.tile([H, 1], FP, tag=nm)
                (nc.sync.dma_start if nm == "b1" else dma)(
                    bt[:], bd.rearrange("(p o) -> p o", o=1))
                bias[nm] = bt
            qc = cpool.tile([H, QW], FP, tag="qc", name="qc")
            dma(qc[:], qc_d[:])

            scr = [dpool.tile([5, ns // NQ], FP, tag=f"scr{h}", name=f"scr{h}")
                   for h in range(NQ)]

            # ---------------- QP (sample-grid layout, fp32, batched) --------
            QS = {}

            def T(hh, tag, w):
                t = tpool.tile([128, w], FP, tag=tag, name=tag)
                QS[hh][tag] = t[:]
                return t[:]

            def qp_pre(hh):
                """x-side preamble: needs only x_d — emitted early, runs on
                idle DVE during the fc1/fc21/fc22 phases."""
                QS[hh] = {}
                g0 = hh * (ns // NQ)
                xg = qpool.tile([128, J6], FP, tag="xg", name="xg")
                nc.sync.dma_start(xg[:],
                    x_d[g0 : g0 + ns // NQ, :].rearrange("(p j) f -> p (j f)", p=128))
                V = nc.vector
                xgv = xg.rearrange("p (j g e) -> p e g j", g=3, e=2)
                x0 = T(hh, "x0", J6)
                x0v = x0.rearrange("p (e g j) -> p e g j", e=2, g=3)
                qsv = qc[:, 0:J6].rearrange("p (e g j) -> p e g j", e=2, g=3)
                qmv = qc[:, J6 : 2 * J6].rearrange("p (e g j) -> p e g j", e=2, g=3)
                V.tensor_mul(x0v, xgv, qsv)
                V.tensor_add(x0v, x0v, qmv)
                dd, vv = x0[:, 0:J3], x0[:, J3:J6]
                d2 = T(hh, "d2", J3); V.tensor_mul(d2, dd, dd)
                d3 = T(hh, "d3", J3); V.tensor_mul(d3, d2, dd)
                d4 = T(hh, "d4", J3); V.tensor_mul(d4, d2, d2)
                v2 = T(hh, "v2", J3); V.tensor_mul(v2, vv, vv)
                dv = T(hh, "dv", J3); V.tensor_mul(dv, d3, vv)
                dw = T(hh, "dw", J3); V.tensor_mul(dw, d2, v2)
                d6 = T(hh, "d6", J3); V.tensor_mul(d6, d3, d3)

                def a3(t, k):
                    return t[:, k * jh : (k + 1) * jh]

                def sum3(t, tag, bias_const=None):
                    r = T(hh, tag, jh)
                    V.tensor_add(r, a3(t, 0), a3(t, 1))
                    if bias_const is None:
                        V.tensor_add(r, r, a3(t, 2))
                    else:
                        V.scalar_tensor_tensor(r, r, bias_const, a3(t, 2),
                                               OP.add, OP.add)
                    return r

                bar = sum3(d4, "bar", -2401.0)   # barrier
                bd = sum3(dv, "bd")              # barrier_dot / 4
                Ls = sum3(dw, "Ls")              # Lf2b / 12
                g6 = sum3(d6, "g6")              # GG / 16
                rg = T(hh, "rg", jh); V.reciprocal(rg, g6)

            def qp_post(hh):
                """head-side chain: needs the scr round-trip."""
                g0 = hh * (ns // NQ)
                V = nc.vector
                q_ = QS[hh]
                hg = qpool.tile([128, 5 * jh], FP, tag="hg", name="hg")
                nc.sync.dma_start(hg.rearrange("p (c j) -> p c j", c=5),
                    scr[hh].rearrange("c (p j) -> p c j", p=128))
                zs = T(hh, "zs", J2)
                V.tensor_add(zs, hg[:, J3 : J3 + J2], qc[:, 2 * J6 + J3 : QW])
                sg = T(hh, "sg", J2)
                nc.scalar.activation(sg, zs, AF.Sigmoid)
                x31v = T(hh, "x31v", J3)
                V.tensor_add(x31v, hg[:, 0:J3], qc[:, 2 * J6 : 2 * J6 + J3])
                d3 = q_["d3"]
                gx = T(hh, "gx", J3); V.tensor_mul(gx, d3, x31v)
                gu = T(hh, "gu", jh)
                V.tensor_add(gu, gx[:, 0:jh], gx[:, jh : 2 * jh])
                V.tensor_add(gu, gu, gx[:, 2 * jh : J3])
                s0t, s1t = sg[:, 0:jh], sg[:, jh:J2]
                ssum = T(hh, "ssum", jh); V.tensor_add(ssum, s0t, s1t)
                sprod = T(hh, "sprod", jh); V.tensor_mul(sprod, s0t, s1t)
                t1 = T(hh, "t1", jh)
                V.scalar_tensor_tensor(t1, ssum, 16.0, q_["bd"], OP.mult, OP.mult)
                t2 = T(hh, "t2", jh)
                V.scalar_tensor_tensor(t2, sprod, 16.0, q_["bar"], OP.mult, OP.mult)
                qa = T(hh, "qa", jh)
                V.scalar_tensor_tensor(qa, gu, 4.0, t1, OP.mult, OP.subtract)
                qb = T(hh, "qb", jh)
                V.scalar_tensor_tensor(qb, q_["Ls"], 12.0, t2, OP.mult, OP.add)
                q = T(hh, "q", jh); V.tensor_sub(q, qa, qb)
                V.tensor_scalar_max(q, q, 0.0)
                lam = T(hh, "lam", jh); V.tensor_mul(lam, q, q_["rg"])
                ui = qpool.tile([128, 3 * jh], FP, tag="ui", name="ui")
                uiv = ui.rearrange("p (j c) -> p c j", c=3)
                for a in range(3):
                    w = T(hh, f"w_{a}", jh)
                    V.tensor_mul(w, lam, d3[:, a * jh : (a + 1) * jh])
                    V.scalar_tensor_tensor(uiv[:, a, :], w, 0.25,
                                           x31v[:, a * jh : (a + 1) * jh],
                                           OP.mult, OP.subtract)
                nc.sync.dma_start(
                    u_d[g0 : g0 + ns // NQ, :].rearrange("(p j) c -> p (j c)", p=128),
                    ui[:])

            # ---------------- MLP: layer-outer phases ----------------
            # All of a layer's inputs are ready before its chunks run, so the
            # ACT stream never stalls on the layer chain.
            hT_all = cpool.tile([H, ns], BF, tag="hT_all", name="hT_all")
            x21a = cpool.tile([H, ns], BF, tag="x21a", name="x21a")
            x22a = cpool.tile([H, ns], BF, tag="x22a", name="x22a")

            def chunk_layer(lhsT, rhs_sl, bias_t, out_sl):
                ps = ppool.tile([128, SC], FP, tag="ps", name="ps")
                for m in range(SC // 512):
                    nc.tensor.matmul(
                        ps[:, 512 * m : 512 * (m + 1)],
                        lhsT,
                        rhs_sl[:, 512 * m : 512 * (m + 1)],
                        start=True, stop=True,
                    )
                nc.scalar.activation(out_sl, ps[:], AF.Tanh, bias=bias_t[:, 0:1])

            def csl(t, i):
                return t[:, SC * i : SC * (i + 1)]

            for i in range(nit):
                xt_c = apool.tile([NF, SC], BF, tag="xt_c", name="xt_c")
                nc.sync.dma_start(xt_c[:], xt_d[:, SC * i : SC * (i + 1)])
                chunk_layer(w1T[:], xt_c[:], bias["b1"], csl(hT_all, i))
            qp_pre(0)
            qp_pre(1)
            for i in range(nit):
                chunk_layer(wts["w21"][:], csl(hT_all, i), bias["b21"], csl(x21a, i))
                chunk_layer(wts["w22"][:], csl(hT_all, i), bias["b22"], csl(x22a, i))

            x21b = cpool.tile([H, ns], BF, tag="x21b", name="x21b")
            for i in range(nit):
                chunk_layer(wts["wm1"][:], csl(x21a, i), bias["bm1"], csl(x21b, i))

            def heads(i):
                s0 = SC * i
                x21 = csl(x21b, i)
                x22 = x22t[i]
                psH = ppool.tile([128, SC], FP, tag="ps", name="ps")
                ps31, ps32 = psH[:, 0:512], psH[:, 512:1024]
                for m in range(4):
                    nc.tensor.matmul(ps31[32 * m : 32 * m + 32, :], wh31[:],
                                     x21[:, 512 * m : 512 * (m + 1)],
                                     start=True, stop=True, tile_position=(0, 32 * m))
                for m in range(4):
                    nc.tensor.matmul(ps32[32 * m : 32 * m + 32, :], wh32[:],
                                     x22[:][:, 512 * m : 512 * (m + 1)],
                                     start=True, stop=True, tile_position=(0, 32 * m))
                hd = hpool.tile([128, 1024], FP, tag="hd", name="hd")
                nc.vector.tensor_copy(hd[:, 0:512], ps31)
                nc.vector.tensor_copy(hd[:, 512:1024], ps32)
                half, off = divmod(s0, ns // NQ)
                for m in range(4):
                    sl = slice(off + 512 * m, off + 512 * (m + 1))
                    eng = nc.sync.dma_start if m % 2 == 0 else nc.gpsimd.dma_start
                    eng(scr[half][0:3, sl], hd[32 * m : 32 * m + 3, 0:512])
                    eng(scr[half][3:5, sl], hd[32 * m : 32 * m + 2, 512:1024])

            x22t = {}
            for i in range(nit):
                x22 = apool.tile([H, SC], BF, tag="x22b", name="x22b")
                chunk_layer(wts["wm2"][:], csl(x22a, i), bias["bm2"], x22[:])
                x22t[i] = x22
                if i > 0:
                    heads(i - 1)
                if i == nit - 1:
                    qp_post(0)
            heads(nit - 1)
            qp_post(1)

    nc.compile()
    return nc


def _get_nc(ns=NS):
    if ns not in _cache:
        _cache[ns] = build(ns)
    return _cache[ns]


def prep_maps(inputs, ns=NS, n_cores=N_CORES):
    """Host-side shard + layout prep. Returns per-core in_maps."""
    f32 = np.float32
    jh = ns // 2 // 128
    g = {k: np.asarray(v) for k, v in inputs.items()}
    x = np.ascontiguousarray(g["x"], f32)
    mean = np.asarray(g["mean"], f32)
    std = np.asarray(g["std"], f32)
    obs = np.array([10.0, 0.0, 10.0, 0.0, 9.0, 0.0], f32)
    moff = mean - obs
    perm = [0, 2, 4, 1, 3, 5]  # pos-block | vel-block order
    qc = np.concatenate([
        np.repeat(std[perm], jh),
        np.repeat(moff[perm], jh),
        np.repeat(np.asarray(g["fc31_b"], f32), jh),
        np.repeat(np.asarray(g["fc32_b"], f32), jh),
    ])
    qc = np.ascontiguousarray(np.broadcast_to(qc, (H, qc.size)))

    def padT(w, cols):
        out = np.zeros((H, 32), f32)
        out[:, :cols] = np.asarray(w, f32).T
        return np.ascontiguousarray(out.astype(BF_NP))

    shared = {
        "w1T": np.ascontiguousarray(np.asarray(g["fc1_w"], f32).T.astype(BF_NP)),
        "w21T": np.ascontiguousarray(np.asarray(g["fc21_w"], f32).T.astype(BF_NP)),
        "w22T": np.ascontiguousarray(np.asarray(g["fc22_w"], f32).T.astype(BF_NP)),
        "wm1T": np.ascontiguousarray(np.asarray(g["fcm1_w"], f32).T.astype(BF_NP)),
        "wm2T": np.ascontiguousarray(np.asarray(g["fcm2_w"], f32).T.astype(BF_NP)),
        "wh31": padT(g["fc31_w"], 3),
        "wh32": padT(g["fc32_w"], 2),
        "b1": np.ascontiguousarray(np.asarray(g["fc1_b"], f32)),
        "b21": np.ascontiguousarray(np.asarray(g["fc21_b"], f32)),
        "b22": np.ascontiguousarray(np.asarray(g["fc22_b"], f32)),
        "bm1": np.ascontiguousarray(np.asarray(g["fcm1_b"], f32)),
        "bm2": np.ascontiguousarray(np.asarray(g["fcm2_b"], f32)),
        "qc": qc,
    }
    in_maps = []
    for c in range(n_cores):
        sh = x[c * ns : (c + 1) * ns]
        m = dict(shared)
        m["x"] = np.ascontiguousarray(sh)
        m["xt"] = np.ascontiguousarray(sh.T.astype(BF_NP))
        in_maps.append(m)
    return in_maps


def kernel(**inputs):
    nc = _get_nc()
    in_maps = prep_maps(inputs)
    res = bass_utils.run_bass_kernel_spmd(nc, in_maps, core_ids=list(range(N_CORES)))
    return np.concatenate([res.results[c]["u"] for c in range(N_CORES)], axis=0)



# revision 30
# speedup vs baseline: 1.0043x; 1.0043x over previous
"""BarrierNet Trainium2 kernel: MLP (6->128->128x2 branches->heads) + closed-form QP.

Data-parallel over 8 cores (16384 samples each). v2 layout:
  - MLP in transposed layout (hidden on partitions, batch free), 8 chunks of
    2048; each layer = 4 bf16 N=512 matmuls into a 4-bank PSUM tile.
  - tanh split across engines: ACT drains cols [0, SC-xoff) with fused
    bias+tanh; the remaining xoff cols go Pool (bias-copy PSUM->SBUF bf16)
    then DVE (clamped odd-poly tanh, all ops in bf16 4x/2x DVE perf modes).
    Only error-insensitive layers (fc1, fc21, fcm1) are offloaded.
  - Heads: per chunk, ps31+ps32 accumulate into ONE PSUM bank (x31 rows
    32m+0..2, z32/2 rows 32m+3..4, 0.5 folded into wh32 host-side), DVE
    drains to SBUF, one SBUF->SBUF DMA scatters into per-half sample-grid
    tiles [128, 5*jh]. No DRAM round-trip.
  - sigmoid via tanh: 4*sig(z) = 2+2*tanh(z/2) -> single ACT table, QP tail
    algebra rewritten in terms of t0,t1 = tanh(z/2).
  - All DMAs issue from SP (HWDGE); Pool does only bias-copies.
"""
import sys

sys.path.insert(0, "/opt/trn_rl_repo")

import numpy as np
import ml_dtypes

import concourse.bacc as bacc
import concourse.bass as bass
import concourse.mybir as mybir
import concourse.tile as tile
from concourse import bass_utils

FP = mybir.dt.float32
BF = mybir.dt.bfloat16
AF = mybir.ActivationFunctionType
OP = mybir.AluOpType
BF_NP = ml_dtypes.bfloat16

N_CORES = 8
B = 131072
NS = B // N_CORES          # samples per core
SC = 2048                  # chunk (one PSUM tile span)
H = 128
NF = 6
NIT = NS // SC
NQ = 2
JH = NS // NQ // 128       # samples per partition in one QP grid half
J3, J2, J6 = 3 * JH, 2 * JH, 6 * JH
QW = 2 * J6 + J3 + J2

# tanh offload widths (cols of each 2048 chunk done by Pool+DVE poly)
X1, X2, X3 = 512, 512, 512

# clamped deg-5 poly: tanh(x) ~ clamp(x*((g*x^2+d)^2+e), -1, 1)
PC2 = 0.008226487
PAL = -8.014924
PBE = 43.06224
PG = float(np.sqrt(PC2))
PD = float(PAL * np.sqrt(PC2))
PE2 = float(PBE * PC2)

_cache = {}


def build(ns=NS):
    nc = bacc.Bacc("TRN2", target_bir_lowering=False, debug=False)

    x_d = nc.dram_tensor("x", [ns, NF], FP, kind="ExternalInput")
    xt_d = nc.dram_tensor("xt", [NF, ns], BF, kind="ExternalInput")
    w1T_d = nc.dram_tensor("w1T", [NF, H], BF, kind="ExternalInput")
    wpack_d = nc.dram_tensor("wpack", [H, 576], BF, kind="ExternalInput")
    fpack_d = nc.dram_tensor("fpack", [H, 5 + QW], FP, kind="ExternalInput")
    u_d = nc.dram_tensor("u", [ns, 3], FP, kind="ExternalOutput")

    with tile.TileContext(nc) as tc:
        with (
            tc.tile_pool(name="const", bufs=1) as cpool,
            tc.tile_pool(name="act", bufs=2) as apool,
            tc.tile_pool(name="xb", bufs=2) as xpool,
            tc.tile_pool(name="hd", bufs=2) as hpool,
            tc.tile_pool(name="psum", bufs=1, space="PSUM") as ppool,
            tc.tile_pool(name="qp", bufs=2) as qpool,
            tc.tile_pool(name="qtmp", bufs=2) as tpool,
            tc.tile_pool(name="dram", bufs=1, space="DRAM") as dpool,
        ):
            scr_d = [dpool.tile([32, ns // NQ], FP, tag=f"scr{h}", name=f"scr{h}")
                     for h in range(NQ)]
            # ---------------- loads (all SP/HWDGE) --------------------------
            # dummy activation on a memset tile: pulls the ACT table load off
            # the critical path (runs at t~0, before any DMA lands)
            dum = cpool.tile([1, 2], FP, tag="dum", name="dum")
            nc.gpsimd.memset(dum[:], 0.0)
            nc.scalar.activation(dum[:], dum[:], AF.Tanh)

            w1T = cpool.tile([NF, H], BF, tag="w1T", name="w1T")
            nc.sync.dma_start(w1T[:], w1T_d[:])
            fpack = cpool.tile([H, 5 + QW], FP, tag="fpack", name="fpack")
            nc.sync.dma_start(fpack[:], fpack_d[:])
            wpack = cpool.tile([H, 576], BF, tag="wpack", name="wpack")
            nc.sync.dma_start(wpack[:], wpack_d[:])

            w21T = wpack[:, 0:128]
            w22T = wpack[:, 128:256]
            wm1T = wpack[:, 256:384]
            wm2T = wpack[:, 384:512]
            whp31 = wpack[:, 512:544]
            whp32 = wpack[:, 544:576]
            b1, b21, b22, bm1, bm2 = (fpack[:, i : i + 1] for i in range(5))
            qc = fpack[:, 5 : 5 + QW]

            xg = {}
            for h in range(NQ):
                g0 = h * (ns // NQ)
                t = cpool.tile([128, J6], FP, tag=f"xg{h}", name=f"xg{h}")
                nc.sync.dma_start(
                    t[:],
                    x_d[g0 : g0 + ns // NQ, :].rearrange(
                        "(p j) f -> p (j f)", p=128))
                xg[h] = t

            gA = [cpool.tile([128, 5 * JH], FP, tag=f"gA{h}", name=f"gA{h}")
                  for h in range(NQ)]

            # ---------------- QP (sample-grid layout, fp32, batched) --------
            QS = {}

            def T(hh, tag, w):
                t = tpool.tile([128, w], FP, tag=tag, name=tag)
                QS[hh][tag] = t[:]
                return t[:]

            def qp_pre(hh):
                QS[hh] = {}
                V = nc.gpsimd    # all-SBUF: legal on Pool, frees DVE
                xgv = xg[hh].rearrange("p (j g e) -> p e g j", g=3, e=2)
                x0 = T(hh, "x0", J6)
                x0v = x0.rearrange("p (e g j) -> p e g j", e=2, g=3)
                qsv = qc[:, 0:J6].rearrange("p (e g j) -> p e g j", e=2, g=3)
                qmv = qc[:, J6 : 2 * J6].rearrange("p (e g j) -> p e g j", e=2, g=3)
                V.tensor_mul(x0v, xgv, qsv)
                V.tensor_add(x0v, x0v, qmv)
                dd, vv = x0[:, 0:J3], x0[:, J3:J6]
                d2 = T(hh, "d2", J3); V.tensor_mul(d2, dd, dd)
                d3 = T(hh, "d3", J3); V.tensor_mul(d3, d2, dd)
                d4 = T(hh, "d4", J3); V.tensor_mul(d4, d2, d2)
                v2 = T(hh, "v2", J3); V.tensor_mul(v2, vv, vv)
                dv = T(hh, "dv", J3); V.tensor_mul(dv, d3, vv)
                dw = T(hh, "dw", J3); V.tensor_mul(dw, d2, v2)
                d6 = T(hh, "d6", J3); V.tensor_mul(d6, d3, d3)

                def a3(t, k):
                    return t[:, k * JH : (k + 1) * JH]

                def sum3(t, tag, bias_const=None):
                    r = T(hh, tag, JH)
                    V.tensor_add(r, a3(t, 0), a3(t, 1))
                    V.tensor_add(r, r, a3(t, 2))
                    if bias_const is not None:
                        V.tensor_scalar(r, r, bias_const, None, OP.add)
                    return r

                bar = sum3(d4, "bar", -2401.0)   # barrier
                bd = sum3(dv, "bd")              # barrier_dot / 4
                Ls = sum3(dw, "Ls")              # Lf2b / 12
                g6 = sum3(d6, "g6")              # GG / 16
                rg = T(hh, "rg", JH); nc.vector.reciprocal(rg, g6)

            def qp_post(hh):
                V = nc.vector   # STT not supported on Pool
                q_ = QS[hh]
                g = gA[hh]
                x31v = T(hh, "x31v", J3)
                V.tensor_add(x31v, g[:, 0:J3], qc[:, 2 * J6 : 2 * J6 + J3])
                zs = T(hh, "zs", J2)
                V.tensor_add(zs, g[:, J3 : J3 + J2], qc[:, 2 * J6 + J3 : QW])
                tt = T(hh, "tt", J2)
                nc.scalar.activation(tt, zs, AF.Tanh)   # t = tanh(z/2)
                t0, t1 = tt[:, 0:JH], tt[:, JH:J2]
                S = T(hh, "S", JH); V.tensor_add(S, t0, t1)
                P = T(hh, "P", JH); V.tensor_mul(P, t0, t1)
                d3 = q_["d3"]
                gx = T(hh, "gx", J3); V.tensor_mul(gx, d3, x31v)
                gu = T(hh, "gu", JH)
                V.tensor_add(gu, gx[:, 0:JH], gx[:, JH : 2 * JH])
                V.tensor_add(gu, gu, gx[:, 2 * JH : J3])
                # lamnum = 4gu - 12Ls - 8(S+2)bd - 4(1+S+P)bar
                tb = T(hh, "tb", JH)
                V.scalar_tensor_tensor(tb, P, 1.0, S, OP.add, OP.add)
                B2 = T(hh, "B2", JH)
                V.scalar_tensor_tensor(B2, tb, -4.0, q_["bar"], OP.mult, OP.mult)
                A2 = T(hh, "A2", JH)
                V.scalar_tensor_tensor(A2, S, 2.0, q_["bd"], OP.add, OP.mult)
                q1 = T(hh, "q1", JH)
                V.scalar_tensor_tensor(q1, A2, -8.0, B2, OP.mult, OP.add)
                q2 = T(hh, "q2", JH)
                V.scalar_tensor_tensor(q2, q_["Ls"], -12.0, q1, OP.mult, OP.add)
                q = T(hh, "q", JH)
                V.scalar_tensor_tensor(q, gu, 4.0, q2, OP.mult, OP.add)
                V.tensor_scalar_max(q, q, 0.0)
                lam = T(hh, "lam", JH); V.tensor_mul(lam, q, q_["rg"])
                ui = qpool.tile([128, 3 * JH], FP, tag="ui", name="ui")
                uiv = ui.rearrange("p (j c) -> p c j", c=3)
                for a in range(3):
                    w = T(hh, f"w_{a}", JH)
                    V.tensor_mul(w, lam, d3[:, a * JH : (a + 1) * JH])
                    V.scalar_tensor_tensor(uiv[:, a, :], w, 0.25,
                                           x31v[:, a * JH : (a + 1) * JH],
                                           OP.mult, OP.subtract)
                g0 = hh * (ns // NQ)
                nc.sync.dma_start(
                    u_d[g0 : g0 + ns // NQ, :].rearrange("(p j) c -> p (j c)", p=128),
                    ui[:])

            # ---------------- PSUM: one manually-windowed tensor ------------
            # fc1/fc2/wm1: alternating [0:2048)/[2048:4096) windows.
            # wm2: 1024-wide 3-slot ring over [0:3072); heads: 512-wide
            # 2-slot ring over [3072:4096). Subtile dep tracking keeps
            # disjoint windows independent.
            psall = ppool.tile([128, 4096], FP, tag="psall", name="psall")
            pcnt = [0]

            # ---------------- MLP chunk helper ------------------------------
            def mlp_chunk(lhsT, rhs_sl, bias_ap, out_sl, xoff, width=SC):
                w0 = (pcnt[0] % 2) * 2048 if width == SC else 1024 * (pcnt[0] % 3)
                pcnt[0] += 1
                ps = psall[:, w0 : w0 + width]
                for m in range(width // 512):
                    nc.tensor.matmul(
                        ps[:, 512 * m : 512 * (m + 1)],
                        lhsT,
                        rhs_sl[:, 512 * m : 512 * (m + 1)],
                        start=True, stop=True,
                    )
                keep = width - xoff
                nc.scalar.activation(out_sl[:, 0:keep], ps[:, 0:keep],
                                     AF.Tanh, bias=bias_ap)
                if xoff:
                    V = nc.vector
                    xb = xpool.tile([128, xoff], BF, tag="xb", name="xb")
                    V.tensor_scalar(xb[:], ps[:, keep:width], bias_ap,
                                    None, OP.add)
                    ta = xpool.tile([128, xoff], BF, tag="ta", name="ta")
                    V.tensor_mul(ta[:], xb[:], xb[:])                 # s = x^2
                    V.tensor_scalar(ta[:], ta[:], PG, PD, OP.mult, OP.add)
                    V.tensor_mul(ta[:], ta[:], ta[:])                 # (gs+d)^2
                    V.tensor_scalar(ta[:], ta[:], PE2, None, OP.add)
                    tb_ = xpool.tile([128, xoff], BF, tag="tb", name="tb")
                    V.tensor_mul(tb_[:], ta[:], xb[:])
                    V.tensor_scalar(out_sl[:, keep:width], tb_[:], 1.0, -1.0,
                                    OP.min, OP.max)

            def csl(t, i):
                return t[:, SC * i : SC * (i + 1)]

            # ---------------- layers ---------------------------------------
            hT_all = cpool.tile([H, ns], BF, tag="hT_all", name="hT_all")
            x21a = cpool.tile([H, ns], BF, tag="x21a", name="x21a")
            x22a = cpool.tile([H, ns], BF, tag="x22a", name="x22a")
            x21b = cpool.tile([H, ns], BF, tag="x21b", name="x21b")

            for i in range(NIT):
                xt_c = apool.tile([NF, SC], BF, tag="xt_c", name="xt_c")
                nc.sync.dma_start(xt_c[:], xt_d[:, SC * i : SC * (i + 1)])
                mlp_chunk(w1T[:], xt_c[:], b1, csl(hT_all, i), X1)
            qp_pre(0)
            qp_pre(1)
            for i in range(NIT):
                mlp_chunk(w21T, csl(hT_all, i), b21, csl(x21a, i), X2)
                mlp_chunk(w22T, csl(hT_all, i), b22, csl(x22a, i), 0)
            for i in range(NIT):
                mlp_chunk(wm1T, csl(x21a, i), bm1, csl(x21b, i), X3)

            def heads(j):
                hw0 = 3072 + 512 * (j % 2)
                psH = psall[:, hw0 : hw0 + 512]
                x21 = csl(x21b, j)
                x22 = x22t[j]
                for m in range(4):
                    nc.tensor.matmul(psH[32 * m : 32 * m + 32, :], whp31,
                                     x21[:, 512 * m : 512 * (m + 1)],
                                     start=True, stop=False,
                                     tile_position=(0, 32 * m))
                    nc.tensor.matmul(psH[32 * m : 32 * m + 32, :], whp32,
                                     x22[:][:, 512 * m : 512 * (m + 1)],
                                     start=False, stop=True,
                                     tile_position=(0, 32 * m))
                hdp = hpool.tile([128, 512], FP, tag="hdp", name="hdp")
                nc.vector.tensor_copy(hdp[:], psH[:])
                h, P0 = divmod(j, 4)
                col0 = 2048 * P0
                for m in range(4):
                    nc.sync.dma_start(
                        scr_d[h][:, col0 + 512 * m : col0 + 512 * (m + 1)],
                        hdp[32 * m : 32 * m + 32, :])
                if P0 == 3:   # half complete: gather the sample-grid tile
                    nc.sync.dma_start(
                        gA[h].rearrange("p (c j) -> p c j", c=5),
                        scr_d[h][0:5, :].rearrange("c (p j) -> p c j", p=128))

            x22t = {}
            pcnt[0] = 0   # wm2 phase: fresh 1024-wide 3-slot ring
            for i in range(NIT):
                x22 = apool.tile([H, SC], BF, tag="x22b", name="x22b")
                for h2 in range(2):
                    mlp_chunk(wm2T, csl(x22a, i)[:, 1024 * h2 : 1024 * (h2 + 1)],
                              bm2, x22[:, 1024 * h2 : 1024 * (h2 + 1)], 0,
                              width=1024)
                x22t[i] = x22
                if i > 0:
                    heads(i - 1)
                if i == 4:
                    qp_post(0)
            heads(NIT - 1)
            qp_post(1)

    nc.compile()
    return nc


def _get_nc(ns=NS):
    if ns not in _cache:
        _cache[ns] = build(ns)
    return _cache[ns]


def prep_maps(inputs, ns=NS, n_cores=N_CORES):
    """Host-side shard + layout prep. Returns per-core in_maps."""
    f32 = np.float32
    jh = ns // NQ // 128
    g = {k: np.asarray(v) for k, v in inputs.items()}
    x = np.ascontiguousarray(g["x"], f32)
    mean = np.asarray(g["mean"], f32)
    std = np.asarray(g["std"], f32)
    obs = np.array([10.0, 0.0, 10.0, 0.0, 9.0, 0.0], f32)
    moff = mean - obs
    perm = [0, 2, 4, 1, 3, 5]  # pos-block | vel-block order
    qc = np.concatenate([
        np.repeat(std[perm], jh),
        np.repeat(moff[perm], jh),
        np.repeat(np.asarray(g["fc31_b"], f32), jh),
        np.repeat(np.asarray(g["fc32_b"], f32) * 0.5, jh),
    ])

    def padT(w, cols, scale=1.0, row0=0):
        out = np.zeros((H, 32), f32)
        out[:, row0 : row0 + cols] = np.asarray(w, f32).T * scale
        return out

    wpack = np.concatenate([
        np.asarray(g["fc21_w"], f32).T,
        np.asarray(g["fc22_w"], f32).T,
        np.asarray(g["fcm1_w"], f32).T,
        np.asarray(g["fcm2_w"], f32).T,
        padT(g["fc31_w"], 3),
        padT(g["fc32_w"], 2, scale=0.5, row0=3),
    ], axis=1)
    wpack = np.ascontiguousarray(wpack.astype(BF_NP))

    fpack = np.concatenate([
        np.stack([np.asarray(g[k], f32) for k in
                  ("fc1_b", "fc21_b", "fc22_b", "fcm1_b", "fcm2_b")], axis=1),
        np.broadcast_to(qc, (H, qc.size)),
    ], axis=1)
    fpack = np.ascontiguousarray(fpack, f32)

    shared = {
        "w1T": np.ascontiguousarray(np.asarray(g["fc1_w"], f32).T.astype(BF_NP)),
        "wpack": wpack,
        "fpack": fpack,
    }
    in_maps = []
    for c in range(n_cores):
        sh = x[c * ns : (c + 1) * ns]
        m = dict(shared)
        m["x"] = np.ascontiguousarray(sh)
        m["xt"] = np.ascontiguousarray(sh.T.astype(BF_NP))
        in_maps.append(m)
    return in_maps


def kernel(**inputs):
    nc = _get_nc()
    in_maps = prep_maps(inputs)
    res = bass_utils.run_bass_kernel_spmd(nc, in_maps, core_ids=list(range(N_CORES)))
    return np.concatenate([res.results[c]["u"] for c in range(N_CORES)], axis=0)
